# revision 18
# baseline (speedup 1.0000x reference)
"""Trainium2 Bass kernel for nn_AttentionNN (8-core SPMD, data-parallel over batch).

Math (per batch b, s=16 sims, F=G=2048):
    A[f,g]   = sum_s X[s,f] Y[s,g]                 (X = data batch, Y = attention batch)
    ls(A)    = A - LSE[g],  LSE[g] = log sum_f exp(A[f,g])
    C[f,s]   = sum_g ls(A)[f,g] Y[s,g]
    gate     = sigmoid([C | X^T] @ W^T + b)
    out[i*32+b, f] = gate[f, i] * data[i*32+b, f]

Key reformulation (eliminates the second [F,G]x[G,s] bmm):
    C[f,s] = (X^T M)[f,s] - v[s],  M = Y Y^T,  v = Y @ LSE
    logits[f,i] = (X^T P)[f,i] + beta[i]
        P    = Y Z^T + W2^T          (Z = W1 @ Y, host-precomputed)
        beta = b - Z @ LSE
So on-device work is: A tiles (TensorE), exp+col-sum of A (ScalarE, the
bottleneck), LSE=log(sum), two tiny Gram-style matmuls, one [64,64]x[64,F]
logits matmul, tanh-based sigmoid, and the final elementwise multiply.
"""

import numpy as np

SIMS = 16
B = 32
F = 2048
NCORES = 8
BPC = B // NCORES          # batches per core = 4
GT = F // 128              # g tiles of 128 = 16
NF = F // 512              # f chunks of 512 = 4
SHIFT = 20.0               # constant shift inside exp (range safety); corrected in hb_row
LN_SCALE_LOG2 = 45         # Ln reads sums * 2^-45 to stay inside the HW Ln range
AMP = 1.0

_CACHE = {}


def _build_nc():
    import concourse.bacc as bacc
    import concourse.tile as tile
    from concourse import mybir
    from contextlib import ExitStack

    f32 = mybir.dt.float32
    bf16 = mybir.dt.bfloat16
    AF = mybir.ActivationFunctionType
    Alu = mybir.AluOpType

    nc = bacc.Bacc(trn_type="TRN2")

    def inp(name, shape):
        return nc.declare_dram_parameter(name, list(shape), f32, isOutput=False)[:]

    xs_pad = inp("xs_pad", (128, F))        # X_b at partitions 32b..32b+15
    ys_pad = inp("ys_pad", (128, F))        # Y_b at partitions 32b..32b+15
    xbig = inp("xbig", (65, F))             # rows 16b+i = X_b, row 64 = ones
    dm_half = inp("dm_half", (64, F))       # row 16b+i = 0.5*AMP*data[i*32 + B0 + b]
    yst = inp("yst", (128, GT * 64))        # col t*64+16b+s = Y_b[s, 128t+p]
    zst = inp("zst", (128, GT * 64))        # col t*64+16b+i = Z_b[i, 128t+p]
    w2t_bd = inp("w2t_bd", (64, 64))        # block-diag W2^T
    i64 = inp("i64", (64, 64))              # identity
    mask_bd = inp("mask_bd", (64, 64))      # block-diag ones
    hb_row = inp("hb_row", (1, 64))         # col 16b+i = b[i] - SHIFT*sum_g Z_b[i,g]
    bmask4 = inp("bmask4", (4, 64))         # [b', 16b+i] = (b'==b)
    ones4 = inp("ones4", (4, 1))
    out_d = nc.declare_dram_parameter("out", [64, F], f32, isOutput=True)[:]

    with ExitStack() as ctx:
        tc = ctx.enter_context(tile.TileContext(nc))
        singles = ctx.enter_context(tc.tile_pool(name="singles", bufs=1))
        apool = ctx.enter_context(tc.tile_pool(name="apsum", bufs=2, space="PSUM"))
        spool = ctx.enter_context(tc.tile_pool(name="scratch", bufs=1))

        def load(ap_dram, shape, tag):
            t = singles.tile(list(shape), f32, tag=tag)
            nc.sync.dma_start(out=t[:], in_=ap_dram)
            return t

        xs_sb = load(xs_pad, (128, F), "xs_sb")
        ys_sb = load(ys_pad, (128, F), "ys_sb")
        xb_sb = load(xbig, (65, F), "xb_sb")
        dm_sb = load(dm_half, (64, F), "dm_sb")
        yst_sb = load(yst, (128, GT * 64), "yst_sb")
        zst_sb = load(zst, (128, GT * 64), "zst_sb")
        w2t_sb = load(w2t_bd, (64, 64), "w2t_sb")
        i64_sb = load(i64, (64, 64), "i64_sb")
        mask_sb = load(mask_bd, (64, 64), "mask_sb")
        hbr_sb = load(hb_row, (1, 64), "hbr_sb")
        bm4_sb = load(bmask4, (4, 64), "bm4_sb")
        on4_sb = load(ones4, (4, 1), "on4_sb")

        neg_shift_sb = singles.tile([128, 1], f32)
        nc.vector.memset(neg_shift_sb[:], -SHIFT)
        zero_sb = singles.tile([128, 1], f32)
        nc.vector.memset(zero_sb[:], 0.0)

        sums_sb = singles.tile([128, GT * BPC], f32)   # col = t*BPC + b
        lse_sb = singles.tile([128, GT * BPC], f32)
        pbig_sb = singles.tile([65, 64], f32)          # rows 0-63: masked Pall; row 64: beta
        bt_sb = singles.tile([4, 64], f32)
        tanh_sb = singles.tile([64, F], f32)
        outm_sb = singles.tile([64, F], f32)
        ex_sb = spool.tile([128, F], bf16)             # exp main output (unused values)

        # ---- main loop: A tiles + exp-accumulate (ScalarE-bound) ----
        for t in range(GT):
            for b in range(BPC):
                ps = apool.tile([128, F], f32, tag="A")
                for c in range(NF):
                    nc.tensor.matmul(
                        ps[:, c * 512:(c + 1) * 512],
                        lhsT=ys_sb[32 * b:32 * b + SIMS, t * 128:(t + 1) * 128],
                        rhs=xs_sb[32 * b:32 * b + SIMS, c * 512:(c + 1) * 512],
                        start=True, stop=True,
                        tile_position=(32 * b, 0),
                    )
                nc.scalar.activation(
                    out=ex_sb[:],
                    in_=ps[:],
                    func=AF.Exp,
                    bias=neg_shift_sb[:],
                    scale=1.0,
                    accum_out=sums_sb[:, t * BPC + b:t * BPC + b + 1],
                )

        # ---- LSE = log(sums)  (SHIFT correction folded into half_b) ----
        nc.scalar.activation(out=lse_sb[:], in_=sums_sb[:], func=AF.Ln,
                             bias=zero_sb[:], scale=float(2.0 ** -LN_SCALE_LOG2))

        # ---- Pall = blockdiag(Y_b Z_b^T) + blockdiag(W2^T), masked ----
        pall_ps = apool.tile([64, 64], f32, tag="A")
        for t in range(GT):
            nc.tensor.matmul(
                pall_ps[:],
                lhsT=yst_sb[:, t * 64:(t + 1) * 64],
                rhs=zst_sb[:, t * 64:(t + 1) * 64],
                start=(t == 0), stop=False,
            )
        nc.tensor.matmul(pall_ps[:], lhsT=i64_sb[:], rhs=w2t_sb[:], start=False, stop=True)
        nc.vector.tensor_mul(pbig_sb[0:64, :], pall_ps[:], mask_sb[:])

        # ---- betaT[b', (b,i)] = sum_g LSE_b'[g] Z_b[i,g]; keep diag blocks, reduce ----
        beta_ps = apool.tile([4, 64], f32, tag="A")
        for t in range(GT):
            nc.tensor.matmul(
                beta_ps[:],
                lhsT=lse_sb[:, t * BPC:(t + 1) * BPC],
                rhs=zst_sb[:, t * 64:(t + 1) * 64],
                start=(t == 0), stop=(t == GT - 1),
            )
        nc.vector.tensor_mul(bt_sb[:], beta_ps[:], bm4_sb[:])
        brow_ps = apool.tile([1, 64], f32, tag="A")
        nc.tensor.matmul(brow_ps[:], lhsT=on4_sb[:], rhs=bt_sb[:], start=True, stop=True)
        # pbig row 64 = hb_row - brow  (the beta row; ones row of xbig picks it up)
        nc.vector.tensor_sub(pbig_sb[64:65, :], hbr_sb[:], brow_ps[:])

        # ---- logits and gate ----
        log_ps = apool.tile([64, F], f32, tag="A")
        for c in range(NF):
            nc.tensor.matmul(
                log_ps[:, c * 512:(c + 1) * 512],
                lhsT=pbig_sb[:],
                rhs=xb_sb[:, c * 512:(c + 1) * 512],
                start=True, stop=True,
            )
        # sigmoid(x) = 0.5*(1 + tanh(x/2)); beta included in logits via ones row
        nc.scalar.activation(
            out=tanh_sb[:], in_=log_ps[:], func=AF.Tanh,
            bias=zero_sb[0:64, :], scale=0.5,
        )
        # out = (tanh + 1) * (0.5*data_perm)
        nc.vector.scalar_tensor_tensor(
            out=outm_sb[:], in0=tanh_sb[:], scalar=1.0, in1=dm_sb[:],
            op0=Alu.add, op1=Alu.mult,
        )
        nc.sync.dma_start(out=out_d, in_=outm_sb[:])

    nc.compile()
    return nc


def _shard_inputs(data, attention, W, b):
    """Build per-core input maps (host-side, not timed)."""
    f32 = np.float32
    data = np.ascontiguousarray(data, dtype=f32)
    attention = np.ascontiguousarray(attention, dtype=f32)
    W = np.ascontiguousarray(W, dtype=f32)
    b_vec = np.ascontiguousarray(b, dtype=f32)
    W1, W2 = W[:, :SIMS], W[:, SIMS:]

    Xb = data.reshape(B, SIMS, F)
    Yb = attention.reshape(B, SIMS, F)
    Dperm = data.reshape(SIMS, B, F)             # [i, b_glob, f]
    Z = np.einsum('is,bsg->big', W1, Yb).astype(f32)   # [B, 16, F]

    w2t_bd = np.zeros((64, 64), f32)
    mask_bd = np.zeros((64, 64), f32)
    bmask4 = np.zeros((4, 64), f32)
    for b in range(BPC):
        w2t_bd[16 * b:16 * b + 16, 16 * b:16 * b + 16] = W2.T
        mask_bd[16 * b:16 * b + 16, 16 * b:16 * b + 16] = 1.0
        bmask4[b, 16 * b:16 * b + 16] = 1.0
    i64 = np.eye(64, dtype=f32)
    ones4 = np.ones((4, 1), f32)

    in_maps = []
    for c in range(NCORES):
        B0 = c * BPC
        xs_pad = np.zeros((128, F), f32)
        ys_pad = np.zeros((128, F), f32)
        xs_pad.reshape(BPC, 32, F)[:, :SIMS] = Xb[B0:B0 + BPC]
        ys_pad.reshape(BPC, 32, F)[:, :SIMS] = Yb[B0:B0 + BPC]
        xbig = np.ones((65, F), f32)
        xbig[:64] = data[B0 * SIMS:(B0 + BPC) * SIMS]
        dm_half = np.ascontiguousarray(
            (0.5 * AMP) * Dperm[:, B0:B0 + BPC].transpose(1, 0, 2).reshape(64, F))
        # yst[p, t*64+16b+s] = Y_b[s, 128t+p]
        yst = np.ascontiguousarray(
            Yb[B0:B0 + BPC].reshape(BPC, SIMS, GT, 128).transpose(3, 2, 0, 1).reshape(128, GT * 64))
        zst = np.ascontiguousarray(
            Z[B0:B0 + BPC].reshape(BPC, SIMS, GT, 128).transpose(3, 2, 0, 1).reshape(128, GT * 64))
        lse_off = SHIFT + LN_SCALE_LOG2 * np.log(2.0)
        hb_row = (b_vec[None, :] - lse_off * Z[B0:B0 + BPC].sum(axis=2)
                  ).astype(f32).reshape(1, 64)
        in_maps.append({
            "xs_pad": xs_pad, "ys_pad": ys_pad, "xbig": xbig,
            "dm_half": dm_half, "yst": yst, "zst": zst,
            "w2t_bd": w2t_bd, "i64": i64, "mask_bd": mask_bd,
            "hb_row": hb_row, "bmask4": bmask4, "ones4": ones4,
        })
    return in_maps


def kernel(data, attention, W, b):
    from concourse.bass_utils import run_bass_kernel_spmd

    if "nc" not in _CACHE:
        _CACHE["nc"] = _build_nc()
    nc = _CACHE["nc"]

    in_maps = _shard_inputs(data, attention, W, b)
    res = run_bass_kernel_spmd(nc, in_maps, core_ids=list(range(NCORES))).results

    out = np.empty((B * SIMS, F), np.float32)
    for c in range(NCORES):
        B0 = c * BPC
        o = res[c]["out"].reshape(BPC, SIMS, F)          # [b, i, f]
        out.reshape(SIMS, B, F)[:, B0:B0 + BPC] = o.transpose(1, 0, 2)
    return out


# revision 25
# speedup vs baseline: 1.9639x; 1.9639x over previous
"""Trainium2 Bass kernel for nn_AttentionNN (8-core SPMD, data-parallel over batch).

Math (per batch b, s=16 sims, F=G=2048):
    A[f,g]   = sum_s X[s,f] Y[s,g]                 (X = data batch, Y = attention batch)
    ls(A)    = A - LSE[g],  LSE[g] = log sum_f exp(A[f,g])
    C[f,s]   = sum_g ls(A)[f,g] Y[s,g]
    gate     = sigmoid([C | X^T] @ W^T + b)
    out[i*32+b, f] = gate[f, i] * data[i*32+b, f]

Key reformulation (eliminates the second [F,G]x[G,s] bmm):
    C[f,s] = (X^T M)[f,s] - v[s],  M = Y Y^T,  v = Y @ LSE
    logits[f,i] = (X^T P)[f,i] + beta[i]
        P    = Y Z^T + W2^T          (Z = W1 @ Y, host-precomputed)
        beta = b - Z @ LSE
So on-device work is: A tiles (TensorE), exp+col-sum of A (ScalarE, the
bottleneck), LSE=log(sum), two tiny Gram-style matmuls, one [64,64]x[64,F]
logits matmul, tanh-based sigmoid, and the final elementwise multiply.
"""

import numpy as np

SIMS = 16
B = 32
F = 2048
NCORES = 8
BPC = B // NCORES          # batches per core = 4
GT = F // 128              # g tiles of 128 = 16
NF = F // 512              # f chunks of 512 = 4
SHIFT = 20.0               # constant shift inside exp (range safety); corrected in hb_row
LN_SCALE_LOG2 = 45         # Ln reads sums * 2^-45 to stay inside the HW Ln range
AMP = 1.0

_CACHE = {}


def _build_nc():
    import concourse.bacc as bacc
    import concourse.tile as tile
    from concourse import mybir
    from contextlib import ExitStack

    f32 = mybir.dt.float32
    bf16 = mybir.dt.bfloat16
    AF = mybir.ActivationFunctionType
    Alu = mybir.AluOpType

    nc = bacc.Bacc(trn_type="TRN2")

    def inp(name, shape):
        return nc.declare_dram_parameter(name, list(shape), f32, isOutput=False)[:]

    def inp16(name, shape):
        return nc.declare_dram_parameter(name, list(shape), bf16, isOutput=False)[:]

    # hi/lo bf16 split operands: batch pair g={0,1}, local j={0,1} at partitions 64j..64j+63
    # ys2: rows [Yh; Yl; Yh; Yl], xs2: rows [Xh; Xh; Xl; Xl] -> K=64 matmul == exact fp32 A
    xs2a = inp16("xs2a", (128, F))
    xs2b = inp16("xs2b", (128, F))
    ys2a = inp16("ys2a", (128, F))
    ys2b = inp16("ys2b", (128, F))
    xbh = inp16("xbh", (65, F))             # rows 16b+i = bf16-hi of X_b, row 64 = ones
    xbl = inp16("xbl", (65, F))             # bf16-lo residual, row 64 = zeros
    dm_half = inp("dm_half", (64, F))       # row 16b+i = 0.5*AMP*data[i*32 + B0 + b]
    yst = inp("yst", (128, GT * 64))        # col t*64+16b+s = Y_b[s, 128t+p]
    zst = inp("zst", (128, GT * 64))        # col t*64+16b+i = Z_b[i, 128t+p]
    w2t_bd = inp("w2t_bd", (64, 64))        # block-diag W2^T
    i64 = inp("i64", (64, 64))              # identity
    mask_bd = inp("mask_bd", (64, 64))      # block-diag ones
    hb_row = inp("hb_row", (1, 64))         # col 16b+i = b[i] - SHIFT*sum_g Z_b[i,g]
    bmask4 = inp("bmask4", (4, 64))         # [b', 16b+i] = (b'==b)
    ones4 = inp("ones4", (4, 1))
    out_d = nc.declare_dram_parameter("out", [64, F], f32, isOutput=True)[:]

    with ExitStack() as ctx:
        tc = ctx.enter_context(tile.TileContext(nc))
        singles = ctx.enter_context(tc.tile_pool(name="singles", bufs=1))
        apool = ctx.enter_context(tc.tile_pool(name="apsum", bufs=2, space="PSUM"))
        spool = ctx.enter_context(tc.tile_pool(name="scratch", bufs=1))

        def load(ap_dram, shape, tag):
            t = singles.tile(list(shape), f32, tag=tag)
            nc.sync.dma_start(out=t[:], in_=ap_dram)
            return t

        def load16(ap_dram, shape, tag):
            t = singles.tile(list(shape), bf16, tag=tag)
            nc.sync.dma_start(out=t[:], in_=ap_dram)
            return t

        xs2_sb = [load16(xs2a, (128, F), "xs2a_sb"), load16(xs2b, (128, F), "xs2b_sb")]
        ys2_sb = [load16(ys2a, (128, F), "ys2a_sb"), load16(ys2b, (128, F), "ys2b_sb")]
        xbh_sb = load16(xbh, (65, F), "xbh_sb")
        xbl_sb = load16(xbl, (65, F), "xbl_sb")
        dm_sb = load(dm_half, (64, F), "dm_sb")
        yst_sb = load(yst, (128, GT * 64), "yst_sb")
        zst_sb = load(zst, (128, GT * 64), "zst_sb")
        w2t_sb = load(w2t_bd, (64, 64), "w2t_sb")
        i64_sb = load(i64, (64, 64), "i64_sb")
        mask_sb = load(mask_bd, (64, 64), "mask_sb")
        hbr_sb = load(hb_row, (1, 64), "hbr_sb")
        bm4_sb = load(bmask4, (4, 64), "bm4_sb")
        on4_sb = load(ones4, (4, 1), "on4_sb")

        neg_shift_sb = singles.tile([128, 1], f32)
        nc.vector.memset(neg_shift_sb[:], -SHIFT)
        zero_sb = singles.tile([128, 1], f32)
        nc.vector.memset(zero_sb[:], 0.0)

        sums_sb = singles.tile([128, GT * BPC], f32)   # col = t*BPC + b
        lse_sb = singles.tile([128, GT * BPC], f32)
        pbig_sb = singles.tile([65, 64], f32)          # rows 0-63: masked Pall; row 64: beta
        bt_sb = singles.tile([4, 64], f32)
        tanh_sb = singles.tile([64, F], f32)
        outm_sb = singles.tile([64, F], f32)
        ex_sb = spool.tile([128, F], bf16)             # exp main output (unused values)

        # ---- main loop: A tiles + exp-accumulate (ScalarE-bound) ----
        for t in range(GT):
            for b in range(BPC):
                grp, j = b // 2, b % 2
                ps = apool.tile([128, F], f32, tag="A")
                for c in range(NF):
                    nc.tensor.matmul(
                        ps[:, c * 512:(c + 1) * 512],
                        lhsT=ys2_sb[grp][64 * j:64 * j + 64, t * 128:(t + 1) * 128],
                        rhs=xs2_sb[grp][64 * j:64 * j + 64, c * 512:(c + 1) * 512],
                        start=True, stop=True,
                        tile_position=(64 * j, 0),
                    )
                nc.scalar.activation(
                    out=ex_sb[:],
                    in_=ps[:],
                    func=AF.Exp,
                    bias=neg_shift_sb[:],
                    scale=1.0,
                    accum_out=sums_sb[:, t * BPC + b:t * BPC + b + 1],
                )

        # ---- LSE = log(sums)  (SHIFT correction folded into half_b) ----
        nc.scalar.activation(out=lse_sb[:], in_=sums_sb[:], func=AF.Ln,
                             bias=zero_sb[:], scale=float(2.0 ** -LN_SCALE_LOG2))

        # ---- Pall = blockdiag(Y_b Z_b^T) + blockdiag(W2^T), masked ----
        pall_ps = apool.tile([64, 64], f32, tag="A")
        for t in range(GT):
            nc.tensor.matmul(
                pall_ps[:],
                lhsT=yst_sb[:, t * 64:(t + 1) * 64],
                rhs=zst_sb[:, t * 64:(t + 1) * 64],
                start=(t == 0), stop=False,
            )
        nc.tensor.matmul(pall_ps[:], lhsT=i64_sb[:], rhs=w2t_sb[:], start=False, stop=True)
        nc.vector.tensor_mul(pbig_sb[0:64, :], pall_ps[:], mask_sb[:])

        # ---- betaT[b', (b,i)] = sum_g LSE_b'[g] Z_b[i,g]; keep diag blocks, reduce ----
        beta_ps = apool.tile([4, 64], f32, tag="A")
        for t in range(GT):
            nc.tensor.matmul(
                beta_ps[:],
                lhsT=lse_sb[:, t * BPC:(t + 1) * BPC],
                rhs=zst_sb[:, t * 64:(t + 1) * 64],
                start=(t == 0), stop=(t == GT - 1),
            )
        nc.vector.tensor_mul(bt_sb[:], beta_ps[:], bm4_sb[:])
        brow_ps = apool.tile([1, 64], f32, tag="A")
        nc.tensor.matmul(brow_ps[:], lhsT=on4_sb[:], rhs=bt_sb[:], start=True, stop=True)
        # pbig row 64 = hb_row - brow  (the beta row; ones row of xbig picks it up)
        nc.vector.tensor_sub(pbig_sb[64:65, :], hbr_sb[:], brow_ps[:])

        # ---- split pbig into bf16 hi/lo so the logits matmul avoids slow fp32 MMs ----
        pbh_sb = singles.tile([65, 64], bf16)
        pbl_sb = singles.tile([65, 64], bf16)
        pbr_sb = singles.tile([65, 64], f32)
        nc.vector.tensor_copy(pbh_sb[:], pbig_sb[:])
        nc.vector.tensor_sub(pbr_sb[:], pbig_sb[:], pbh_sb[:])
        nc.vector.tensor_copy(pbl_sb[:], pbr_sb[:])

        # ---- logits and gate: A@B = AhBh + AlBh + AhBl + AlBl in bf16 ----
        log_ps = apool.tile([64, F], f32, tag="A")
        for c in range(NF):
            sl = slice(c * 512, (c + 1) * 512)
            nc.tensor.matmul(log_ps[:, sl], lhsT=pbh_sb[:], rhs=xbh_sb[:, sl],
                             start=True, stop=False)
            nc.tensor.matmul(log_ps[:, sl], lhsT=pbl_sb[:], rhs=xbh_sb[:, sl],
                             start=False, stop=False)
            nc.tensor.matmul(log_ps[:, sl], lhsT=pbh_sb[:], rhs=xbl_sb[:, sl],
                             start=False, stop=False)
            nc.tensor.matmul(log_ps[:, sl], lhsT=pbl_sb[:], rhs=xbl_sb[:, sl],
                             start=False, stop=True)
        # sigmoid(x) = 0.5*(1 + tanh(x/2)); beta included in logits via ones row
        nc.scalar.activation(
            out=tanh_sb[:], in_=log_ps[:], func=AF.Tanh,
            bias=zero_sb[0:64, :], scale=0.5,
        )
        # out = (tanh + 1) * (0.5*data_perm)
        nc.vector.scalar_tensor_tensor(
            out=outm_sb[:], in0=tanh_sb[:], scalar=1.0, in1=dm_sb[:],
            op0=Alu.add, op1=Alu.mult,
        )
        nc.sync.dma_start(out=out_d, in_=outm_sb[:])

    nc.compile()
    return nc


def _shard_inputs(data, attention, W, b):
    """Build per-core input maps (host-side, not timed)."""
    import ml_dtypes
    f32 = np.float32
    bf16 = ml_dtypes.bfloat16

    def hilo(x):
        xh = x.astype(bf16)
        xl = (x - xh.astype(f32)).astype(bf16)
        return xh, xl
    data = np.ascontiguousarray(data, dtype=f32)
    attention = np.ascontiguousarray(attention, dtype=f32)
    W = np.ascontiguousarray(W, dtype=f32)
    b_vec = np.ascontiguousarray(b, dtype=f32)
    W1, W2 = W[:, :SIMS], W[:, SIMS:]

    Xb = data.reshape(B, SIMS, F)
    Yb = attention.reshape(B, SIMS, F)
    Dperm = data.reshape(SIMS, B, F)             # [i, b_glob, f]
    Z = np.einsum('is,bsg->big', W1, Yb).astype(f32)   # [B, 16, F]

    w2t_bd = np.zeros((64, 64), f32)
    mask_bd = np.zeros((64, 64), f32)
    bmask4 = np.zeros((4, 64), f32)
    for b in range(BPC):
        w2t_bd[16 * b:16 * b + 16, 16 * b:16 * b + 16] = W2.T
        mask_bd[16 * b:16 * b + 16, 16 * b:16 * b + 16] = 1.0
        bmask4[b, 16 * b:16 * b + 16] = 1.0
    i64 = np.eye(64, dtype=f32)
    ones4 = np.ones((4, 1), f32)

    in_maps = []
    for c in range(NCORES):
        B0 = c * BPC
        xs2 = [np.zeros((128, F), bf16) for _ in range(2)]
        ys2 = [np.zeros((128, F), bf16) for _ in range(2)]
        for bb in range(BPC):
            grp, j = bb // 2, bb % 2
            Xh, Xl = hilo(Xb[B0 + bb])
            Yh, Yl = hilo(Yb[B0 + bb])
            xs2[grp][64 * j + 0:64 * j + 16] = Xh
            xs2[grp][64 * j + 16:64 * j + 32] = Xh
            xs2[grp][64 * j + 32:64 * j + 48] = Xl
            xs2[grp][64 * j + 48:64 * j + 64] = Xl
            ys2[grp][64 * j + 0:64 * j + 16] = Yh
            ys2[grp][64 * j + 16:64 * j + 32] = Yl
            ys2[grp][64 * j + 32:64 * j + 48] = Yh
            ys2[grp][64 * j + 48:64 * j + 64] = Yl
        xbig = np.ones((65, F), f32)
        xbig[:64] = data[B0 * SIMS:(B0 + BPC) * SIMS]
        xbh_a, xbl_a = hilo(xbig)
        dm_half = np.ascontiguousarray(
            (0.5 * AMP) * Dperm[:, B0:B0 + BPC].transpose(1, 0, 2).reshape(64, F))
        # yst[p, t*64+16b+s] = Y_b[s, 128t+p]
        yst = np.ascontiguousarray(
            Yb[B0:B0 + BPC].reshape(BPC, SIMS, GT, 128).transpose(3, 2, 0, 1).reshape(128, GT * 64))
        zst = np.ascontiguousarray(
            Z[B0:B0 + BPC].reshape(BPC, SIMS, GT, 128).transpose(3, 2, 0, 1).reshape(128, GT * 64))
        lse_off = SHIFT + LN_SCALE_LOG2 * np.log(2.0)
        hb_row = (b_vec[None, :] - lse_off * Z[B0:B0 + BPC].sum(axis=2)
                  ).astype(f32).reshape(1, 64)
        in_maps.append({
            "xs2a": xs2[0], "xs2b": xs2[1], "ys2a": ys2[0], "ys2b": ys2[1],
            "xbh": xbh_a, "xbl": xbl_a,
            "dm_half": dm_half, "yst": yst, "zst": zst,
            "w2t_bd": w2t_bd, "i64": i64, "mask_bd": mask_bd,
            "hb_row": hb_row, "bmask4": bmask4, "ones4": ones4,
        })
    return in_maps


def kernel(data, attention, W, b):
    from concourse.bass_utils import run_bass_kernel_spmd

    if "nc" not in _CACHE:
        _CACHE["nc"] = _build_nc()
    nc = _CACHE["nc"]

    in_maps = _shard_inputs(data, attention, W, b)
    res = run_bass_kernel_spmd(nc, in_maps, core_ids=list(range(NCORES))).results

    out = np.empty((B * SIMS, F), np.float32)
    for c in range(NCORES):
        B0 = c * BPC
        o = res[c]["out"].reshape(BPC, SIMS, F)          # [b, i, f]
        out.reshape(SIMS, B, F)[:, B0:B0 + BPC] = o.transpose(1, 0, 2)
    return out
